# revision 18
# baseline (speedup 1.0000x reference)
"""CoLightNet Trainium2 Bass kernel (self-contained).

SPMD over 8 cores; core c owns output rows [c*1024, (c+1)*1024).
  inputs : state    [N,S]    f32  (replicated)
           state_mb [Mc,S]   f32  (core's own row block, for q)
           adjinvt  [N,Mc]   i8   (transposed inverted adjacency: 1-adj[rows].T)
           w1,w2,wq,wk,wh1 [128,128], wh2 [128,8], biases as [*,1] f32
           (wq is pre-scaled by 1/sqrt(E) on the host)
  output : outb     [Mc,A]   f32

Math (identical to the reference, reformulated):
  h    = relu(state@w1+b1)@w2+b2
  sT   = (h@wk)^T-stationary x (h@wq/sqrt(E))-moving          # scores transposed [n,m]
  wT   = exp(sT - BIG*adjinv)                                 # == exp(s)*adj + O(e^-45)
  aggT = h^T-stationary x wT-moving  (per n-block, accumulated)  # [E, m]
  den  = ones^T x wT  (same accumulation)                     # [1, m]
  out  = relu((aggT/den)^T @ wh1 + bh1) @ wh2 + bh2

All heavy matmuls use float32r (single-pass PE, ~1.6e-4 rel err) with moving
dim 512; the masked-score path stays fp32 until the exp output.
"""

from contextlib import ExitStack

import concourse.bass as bass
import concourse.mybir as mybir
import concourse.tile as tile
from concourse import bacc
from concourse.masks import make_identity

F32 = mybir.dt.float32
F32R = mybir.dt.float32r
I8 = mybir.dt.int8
AF = mybir.ActivationFunctionType
ALU = mybir.AluOpType

S = 128
E = 128
A = 8
BIG = 50.0


def ts(i, size):
    return slice(i * size, (i + 1) * size)


def build_kernel(n_total=8192, m_core=1024):
    nc = bacc.Bacc("TRN2", debug=False)
    state = nc.dram_tensor("state", (n_total, S), F32, kind="ExternalInput").ap()
    state_mb = nc.dram_tensor("state_mb", (m_core, S), F32, kind="ExternalInput").ap()
    adjinvt = nc.dram_tensor("adjinvt", (n_total, m_core), I8, kind="ExternalInput").ap()
    wt = {}
    for name, shape in [
        ("w1", (S, E)), ("w2", (E, E)), ("wq", (E, E)), ("wk", (E, E)),
        ("wh1", (E, E)), ("wh2", (E, A)),
        ("b1", (E, 1)), ("b2", (E, 1)), ("bh1", (E, 1)), ("bh2", (A, 1)),
    ]:
        wt[name] = nc.dram_tensor(name, shape, F32, kind="ExternalInput").ap()
    outb = nc.dram_tensor("outb", (m_core, A), F32, kind="ExternalOutput").ap()

    with tile.TileContext(nc) as tc:
        colight_body(tc, outb, state, state_mb, adjinvt, wt)
    nc.compile()
    return nc


def colight_body(tc, outb, state, state_mb, adjinvt, wt):
    nc = tc.nc
    n_total = state.shape[0]
    m_core = adjinvt.shape[1]
    NT = n_total // 512   # n-groups of 512
    NB = n_total // 128   # n-blocks of 128
    MT = m_core // 512    # m-halves of 512

    with ExitStack() as ctx:
        singles = ctx.enter_context(tc.tile_pool(name="singles", bufs=1))

        # ---- constants ----
        wf = {}
        for name, shape in [("w1", [S, E]), ("w2", [E, E]), ("wq", [E, E]),
                            ("wk", [E, E]), ("wh1", [E, E]), ("wh2", [E, A]),
                            ("b1", [E, 1]), ("b2", [E, 1]), ("bh1", [E, 1]),
                            ("bh2", [A, 1])]:
            t = singles.tile(shape, F32, tag=f"w_{name}")
            nc.scalar.dma_start(out=t, in_=wt[name])
            wf[name] = t
        # fp32r-rounded copies of the MLP weights (fp32r matmul inputs must be
        # produced by a rounding op for the BIR verifier)
        w1r = singles.tile([S, E], F32R)
        w2r = singles.tile([E, E], F32R)
        wqr = singles.tile([E, E], F32R)
        wkr = singles.tile([E, E], F32R)
        for dst, src in [(w1r, "w1"), (w2r, "w2"), (wqr, "wq"), (wkr, "wk")]:
            nc.vector.tensor_copy(out=dst, in_=wf[src])
        ident = singles.tile([128, 128], F32)
        make_identity(nc, ident)
        ones_f = singles.tile([128, 1], F32)
        nc.vector.memset(ones_f, 1.0)
        ones_r = singles.tile([128, 1], F32R)
        nc.vector.tensor_copy(out=ones_r, in_=ones_f)
        identr = singles.tile([128, 128], F32R)
        nc.vector.tensor_copy(out=identr, in_=ident)

        # ---- persistent activations (fp32r matmul operands) ----
        kTs = singles.tile([E, n_total], F32R)
        qTs = singles.tile([E, m_core], F32R)
        hblk = singles.tile([128, NB, E], F32R)    # h in normal orientation
        bufA = singles.tile([128, n_total], F32R)  # stateT, later hT
        bufB = singles.tile([128, n_total], F32R)  # h1T

        ph1_stack = ExitStack()
        ph1_sb = ph1_stack.enter_context(tc.tile_pool(name="ph1_sb", bufs=3))
        ph1_ps = ph1_stack.enter_context(tc.tile_pool(name="ph1_ps", bufs=2, space="PSUM"))

        # ---- phase 1e: mini-MLP on the core's own rows -> qT ----
        for j in range(MT):
            sm_in = ph1_sb.tile([128, 4, S], F32, tag="sb")
            nc.sync.dma_start(
                out=sm_in,
                in_=state_mb[ts(j, 512), :].rearrange("(a p) s -> p a s", p=128),
            )
            tp = ph1_ps.tile([128, 512], F32, tag="p")
            for a in range(4):
                nc.tensor.transpose(tp[:, ts(a, 128)], sm_in[:, a, :], ident)
            smT = ph1_sb.tile([128, 512], F32R, tag="sb2")
            nc.scalar.copy(out=smT, in_=tp)
            ps1 = ph1_ps.tile([128, 512], F32, tag="p")
            nc.tensor.matmul(ps1, w1r, smT, start=True, stop=True)
            h1m = ph1_sb.tile([128, 512], F32R, tag="sb2")
            nc.scalar.activation(h1m, ps1, AF.Relu, bias=wf["b1"], scale=1.0)
            ps2 = ph1_ps.tile([128, 512], F32, tag="p")
            nc.tensor.matmul(ps2, w2r, h1m, start=True, stop=True)
            hm = ph1_sb.tile([128, 512], F32R, tag="sb2")
            nc.scalar.activation(hm, ps2, AF.Identity, bias=wf["b2"], scale=1.0)
            ps3 = ph1_ps.tile([128, 512], F32, tag="p")
            nc.tensor.matmul(ps3, wqr, hm, start=True, stop=True)
            nc.scalar.copy(out=qTs[:, ts(j, 512)], in_=ps3)


        # ---- phase 1a: stateT via PE transposes ----
        for j in range(NT):
            st_in = ph1_sb.tile([128, 4, S], F32, tag="sb")
            nc.sync.dma_start(
                out=st_in,
                in_=state[ts(j, 512), :].rearrange("(a p) s -> p a s", p=128),
            )
            tp = ph1_ps.tile([128, 512], F32, tag="p")
            for a in range(4):
                nc.tensor.transpose(tp[:, ts(a, 128)], st_in[:, a, :], ident)
            nc.scalar.copy(out=bufA[:, ts(j, 512)], in_=tp)

        # ---- phase 1b/c: h1T = relu(w1^T stateT + b1); hT = w2^T h1T + b2 ----
        for j in range(NT):
            ps = ph1_ps.tile([128, 512], F32, tag="p")
            nc.tensor.matmul(ps, w1r, bufA[:, ts(j, 512)], start=True, stop=True)
            nc.scalar.activation(bufB[:, ts(j, 512)], ps, AF.Relu, bias=wf["b1"], scale=1.0)
        for j in range(NT):
            ps = ph1_ps.tile([128, 512], F32, tag="p")
            nc.tensor.matmul(ps, w2r, bufB[:, ts(j, 512)], start=True, stop=True)
            nc.scalar.activation(bufA[:, ts(j, 512)], ps, AF.Identity, bias=wf["b2"], scale=1.0)
        hT = bufA
        # ---- phase 1d: kT, h (normal orientation) ----
        for j in range(NT):
            ps = ph1_ps.tile([128, 512], F32, tag="p")
            nc.tensor.matmul(ps, wkr, hT[:, ts(j, 512)], start=True, stop=True)
            nc.vector.tensor_copy(out=kTs[:, ts(j, 512)], in_=ps)
        for j in range(NT):
            hx = ph1_ps.tile([128, 512], F32R, tag="p")
            for a in range(4):
                nb = j * 4 + a
                nc.tensor.transpose(hx[:, ts(a, 128)], hT[:, ts(nb, 128)], identr)
            nc.vector.tensor_copy(
                out=hblk[:, ts(j, 4), :],
                in_=hx.rearrange("p (a e) -> p a e", a=4),
            )

        ph1_stack.close()

        # ---- phase 2: transposed masked attention, no on-chip transposes ----
        adj_pool = ctx.enter_context(tc.tile_pool(name="adj", bufs=6))
        msk_pool = ctx.enter_context(tc.tile_pool(name="msk", bufs=8))
        expT_pool = ctx.enter_context(tc.tile_pool(name="expT", bufs=8))
        sc_ps = ctx.enter_context(tc.tile_pool(name="sc_ps", bufs=3, space="PSUM"))
        agg_psp = ctx.enter_context(tc.tile_pool(name="agg_ps", bufs=2, space="PSUM"))
        den_psp = ctx.enter_context(tc.tile_pool(name="den_ps", bufs=1, space="PSUM"))
        head_sb = ctx.enter_context(tc.tile_pool(name="head_sb", bufs=2))
        head_ps = ctx.enter_context(tc.tile_pool(name="head_ps", bufs=2, space="PSUM"))

        for mh in range(MT):
            aggT = agg_psp.tile([128, 512], F32)
            den = den_psp.tile([1, 512], F32)
            for ng in range(NT):
                adjt = adj_pool.tile([128, 4, 512], I8)
                nc.sync.dma_start(
                    out=adjt,
                    in_=adjinvt[ts(ng, 512), ts(mh, 512)].rearrange(
                        "(a p) m -> p a m", p=128
                    ),
                )
                for a in range(4):
                    nb = ng * 4 + a
                    scp = sc_ps.tile([128, 512], F32)
                    nc.tensor.matmul(
                        scp, kTs[:, ts(nb, 128)], qTs[:, ts(mh, 512)],
                        start=True, stop=True,
                    )
                    mskT = msk_pool.tile([128, 512], F32)
                    nc.vector.scalar_tensor_tensor(
                        out=mskT,
                        in0=adjt[:, a, :],
                        scalar=-BIG,
                        in1=scp,
                        op0=ALU.mult,
                        op1=ALU.add,
                    )
                    expT = expT_pool.tile([128, 512], F32R)
                    nc.scalar.activation(expT, mskT, AF.Exp, bias=0.0, scale=1.0)
                    nc.tensor.matmul(
                        aggT, hblk[:, nb, :], expT,
                        start=(nb == 0), stop=(nb == NB - 1),
                    )
                    nc.tensor.matmul(
                        den, ones_r, expT,
                        start=(nb == 0), stop=(nb == NB - 1),
                    )
            # ---- normalize + head, per 128-row m-subblock ----
            aggT_sb = head_sb.tile([128, 512], F32, tag="aggsb")
            nc.vector.tensor_copy(out=aggT_sb, in_=aggT)
            den_sb = head_sb.tile([1, 512], F32, tag="densb")
            nc.vector.tensor_copy(out=den_sb, in_=den)
            for q in range(4):
                mb = mh * 4 + q
                denT_ps = head_ps.tile([128, 1], F32, tag="hps")
                nc.tensor.transpose(denT_ps, den_sb[0:1, ts(q, 128)], ident[0:1, 0:1])
                rden = head_sb.tile([128, 1], F32, tag="hsb1")
                nc.vector.reciprocal(rden, denT_ps)
                agq_ps = head_ps.tile([128, 128], F32, tag="hps")
                nc.tensor.transpose(agq_ps, aggT_sb[:, ts(q, 128)], ident)
                aggn = head_sb.tile([128, E], F32, tag="hsb")
                nc.vector.tensor_scalar_mul(out=aggn, in0=agq_ps, scalar1=rden)
                aggnT_ps = head_ps.tile([128, 128], F32, tag="hps")
                nc.tensor.transpose(aggnT_ps, aggn, ident)
                aggnT = head_sb.tile([128, 128], F32, tag="hsb")
                nc.scalar.copy(out=aggnT, in_=aggnT_ps)
                h3_ps = head_ps.tile([128, 128], F32, tag="hps")
                nc.tensor.matmul(h3_ps, wf["wh1"], aggnT, start=True, stop=True)
                h3 = head_sb.tile([128, 128], F32, tag="hsb")
                nc.scalar.activation(h3, h3_ps, AF.Relu, bias=wf["bh1"], scale=1.0)
                oT_ps = head_ps.tile([8, 128], F32, tag="hps")
                nc.tensor.matmul(oT_ps, wf["wh2"], h3, start=True, stop=True)
                oT = head_sb.tile([8, 128], F32, tag="hsb")
                nc.scalar.activation(oT, oT_ps, AF.Identity, bias=wf["bh2"], scale=1.0)
                o_ps = head_ps.tile([128, A], F32, tag="hps")
                nc.tensor.transpose(o_ps, oT, ident[0:8, 0:8])
                o_sb = head_sb.tile([128, A], F32, tag="hsb")
                nc.vector.tensor_copy(out=o_sb, in_=o_ps)
                nc.scalar.dma_start(out=outb[ts(mb, 128), :], in_=o_sb)


# ----------------------------------------------------------------------------
# Host entry point: full inputs in, full output out. 8-way row sharding.
# ----------------------------------------------------------------------------
import numpy as np

N_TOTAL = 8192
N_CORES = 8
M_CORE = N_TOTAL // N_CORES

_cached = {}


def _get_nc():
    if "nc" not in _cached:
        _cached["nc"] = build_kernel(n_total=N_TOTAL, m_core=M_CORE)
    return _cached["nc"]


def make_in_maps(state_matrix, adj, w1, b1, w2, b2, wq, wk, wh1, bh1, wh2, bh2):
    state_matrix = np.ascontiguousarray(np.asarray(state_matrix, dtype=np.float32))
    adj = np.asarray(adj)
    f32 = lambda x: np.ascontiguousarray(np.asarray(x, dtype=np.float32))
    wq_scaled = f32(wq) / np.float32(np.sqrt(E))
    # inverted, transposed adjacency as int8: adjinvt[n, m] = 1 - adj[m, n]
    adjinvt_full = np.ascontiguousarray((adj == 0).T.astype(np.int8))
    common = {
        "state": state_matrix,
        "w1": f32(w1), "w2": f32(w2), "wq": wq_scaled, "wk": f32(wk),
        "wh1": f32(wh1), "wh2": f32(wh2),
        "b1": f32(b1).reshape(E, 1), "b2": f32(b2).reshape(E, 1),
        "bh1": f32(bh1).reshape(E, 1), "bh2": f32(bh2).reshape(A, 1),
    }
    in_maps = []
    for c in range(N_CORES):
        rows = slice(c * M_CORE, (c + 1) * M_CORE)
        in_maps.append(
            dict(
                common,
                state_mb=state_matrix[rows],
                adjinvt=np.ascontiguousarray(adjinvt_full[:, rows]),
            )
        )
    return in_maps


def kernel(state_matrix, adj, w1, b1, w2, b2, wq, wk, wh1, bh1, wh2, bh2):
    from concourse import bass_utils

    in_maps = make_in_maps(
        state_matrix, adj, w1, b1, w2, b2, wq, wk, wh1, bh1, wh2, bh2
    )
    res = bass_utils.run_bass_kernel_spmd(
        _get_nc(), in_maps, core_ids=list(range(N_CORES))
    )
    out = np.concatenate([r["outb"] for r in res.results], axis=0)
    return out.astype(np.float32)
